# revision 57
# baseline (speedup 1.0000x reference)
"""DigitCaps (CapsNet dynamic routing) Bass kernel for Trainium2, 8 NeuronCores.

reference math:
    u_hat[b,r,c,o] = sum_i W[r,c,o,i] * x[b,r,i]     B,R,C,O,I = 256,1152,10,16,8
    b_ij = 0
    for it in 0,1,2:
        c_ij = softmax(b_ij, axis=c)
        s[b,c,o] = sum_r c_ij[b,r,c] * u_hat[b,r,c,o]
        v = squash(s)
        if it < 2: b_ij += sum_o u_hat[b,r,c,o] * v[b,c,o]

Sharding: data-parallel over B (32 batches/core), W replicated. Routing is
batch-local -> no collectives; one SPMD NEFF, per-core input slices.

Per-core mapping (b-tiles of 8 batches; r-blocks of 16):
  * u_hat created on PE as block-diag matmuls:
      lhsT = xbd[(rs,i)=128, (b,rs')=128] (host-packed, zero off-diagonal)
      rhs  = Wp[(rs,i)=128, (c,o)=160]    (host-packed)
      out  = u[(b,rs), (c,o)] per r-block, 3 r-blocks per PSUM bank,
      grouped ACT copies -> SBUF u quarters [128, 18, 160] x4 (per-quarter
      dep granularity lets the routing start before creation finishes)
  * s0 (uniform c=0.1) via dense K=9216 matmul from xk/Wp directly; wp/xk
    loaded as 6 chunk-tiles so PE starts after the first chunk lands.
  * per iteration, pipelined in 4 chunks of 18 r-blocks (DVE->GPSIMD->PE):
      g = sum_o u*v: DVE tensor_tensor mult (v broadcast to [(b,rs),(c,o)]
        via tiny PE matmul against host 0/1 E-matrices) + tensor_reduce
      softmax over c without max-subtraction (logits bounded ~||u||*||v||),
        1/sum via the single-op Newton-Raphson reciprocal_approx_fast
      cbd[(b,rs),(c',b')] block-diag c_ij built by one GPSIMD mask-multiply
      s += cbd.T @ u on PE, accumulating all 72 r-blocks in one PSUM bank
  * t-matrix diagonal c'=c extracted by mask-multiply + reduce into
    s80[(c,b), o] where squash is pure per-partition-scalar work.
  * output v in [(c,b), o] layout, host un-permutes.

TimelineSim cost-model estimate: ~356 us/core (engines: PE 267, DVE 220,
GPSIMD 109, ACT 90, DMA 78). fp32 end-to-end; measured scale-relative error
vs the jax reference: 3.0e-06.
"""

import sys

sys.path.insert(0, "/opt/trn_rl_repo")

import os

import numpy as np

# matmul dtype: "f32" (exact, 4 cyc/row), "f32r" (same bits, replicated-fp32
# PE path), or "bf16" (1 cyc/row, ~1e-3 accurate). Env BASS_DC_MMDT for A/B.
MM_DTYPE = os.environ.get("BASS_DC_MMDT", "f32")

B, R, C, O, I = 256, 1152, 10, 16, 8
NCORES = 8
BL = B // NCORES  # 32 batches per core
NBT, BT = 4, 8  # 4 b-tiles of 8
RB, RS = 72, 16  # 72 r-blocks of 16
CO = C * O  # 160
P = 128


# ----------------------------------------------------------------------------
# host-side packing
# ----------------------------------------------------------------------------
def _mmnp():
    if MM_DTYPE == "bf16":
        import ml_dtypes

        return ml_dtypes.bfloat16
    return np.float32


def _pack_shared(W):
    W = np.asarray(W, dtype=np.float32)
    # Wp[rb, rs*8+i, c*16+o] = W[rb*16+rs, c, o, i]
    wp = (
        W.reshape(RB, RS, C, O, I)
        .transpose(0, 1, 4, 2, 3)
        .reshape(RB, P, CO)
        .copy()
    )
    # E2all[k, bt, b*16+rs] = 1 iff k == bt*8+b   (vbc for iter 1 from v_full)
    e2 = np.zeros((P, NBT, P), np.float32)
    for bt in range(NBT):
        for b in range(BT):
            e2[bt * BT + b, bt, b * RS : (b + 1) * RS] = 1.0
    # E2c[c'*8+b', c, b*16+rs] = 1 iff c'==c and b'==b  (vbc for iter 2 from v80)
    e2c = np.zeros((P, C, P), np.float32)
    for c in range(C):
        for b in range(BT):
            e2c[c * BT + b, c, b * RS : (b + 1) * RS] = 1.0
    # dmask[(b,rs), (c',b')] = 1 iff b'==b : builds block-diag c_ij via mult
    dmask = np.zeros((P, C, BT), np.float32)
    for b in range(BT):
        dmask[b * RS : (b + 1) * RS, :, b] = 1.0
    # extmask[(c',b), (c,o)] = 1 iff c==c' : extracts t-matrix diagonal
    extmask = np.zeros((C * BT, C, O), np.float32)
    for c in range(C):
        extmask[c * BT : (c + 1) * BT, c, :] = 1.0
    mdt = _mmnp()
    return wp.astype(mdt), e2.astype(mdt), e2c.astype(mdt), dmask, extmask


def _pack_core(xc):
    """xc: [32, 1152, 8] slice for one core -> (xbd, xk)."""
    xc = np.asarray(xc, dtype=np.float32)
    # xk[kc, p, b] = xc[b, (r,i)flat = kc*128+p]
    xk = np.ascontiguousarray(
        xc.reshape(BL, R * I).T.reshape(RB, P, BL)
    )
    # xbd[bt, rb, rs*8+i, b*16+rs] = xc[bt*8+b, rb*16+rs, i]
    xbd = np.zeros((NBT, RB, P, P), np.float32)
    xcr = xc.reshape(NBT, BT, RB, RS, I)
    for rs in range(RS):
        xbd[:, :, rs * I : (rs + 1) * I, rs::RS] = xcr[:, :, :, rs, :].transpose(
            0, 2, 3, 1
        )
    mdt = _mmnp()
    return xbd.astype(mdt), xk.astype(mdt)


# ----------------------------------------------------------------------------
# bass module
# ----------------------------------------------------------------------------
_MODULE_CACHE = {}

def _build_module():
    key = ("nc", MM_DTYPE)
    if key in _MODULE_CACHE:
        return _MODULE_CACHE[key]

    import concourse.bass as bass
    import concourse.mybir as mybir
    import concourse.tile as tile
    from concourse import bacc

    f32 = mybir.dt.float32
    mdt = mybir.dt.bfloat16 if MM_DTYPE == "bf16" else f32
    AX = mybir.AxisListType
    ALU = mybir.AluOpType
    ACTF = mybir.ActivationFunctionType

    nc = bacc.Bacc(
        "TRN2",
        target_bir_lowering=False,
        debug=False,
        enable_asserts=False,
    )

    if MM_DTYPE == "f32r":
        _mc = lambda ap: ap.bitcast(mybir.dt.float32r)
    else:
        _mc = lambda ap: ap

    def mm(out, lhsT, rhs, **kw):
        nc.tensor.matmul(out, _mc(lhsT), _mc(rhs), **kw)

    xbd_d = nc.dram_tensor("xbd", [NBT, RB, P, P], mdt, kind="ExternalInput").ap()
    xk_d = nc.dram_tensor("xk", [RB, P, BL], mdt, kind="ExternalInput").ap()
    wp_d = nc.dram_tensor("wp", [RB, P, CO], mdt, kind="ExternalInput").ap()
    e2_d = nc.dram_tensor("e2", [P, NBT, P], mdt, kind="ExternalInput").ap()
    e2c_d = nc.dram_tensor("e2c", [P, C, P], mdt, kind="ExternalInput").ap()
    dmask_d = nc.dram_tensor("dmask", [P, C, BT], f32, kind="ExternalInput").ap()
    extmask_d = nc.dram_tensor("extmask", [C * BT, C, O], f32, kind="ExternalInput").ap()
    vout_d = nc.dram_tensor("vout", [NBT, C * BT, O], f32, kind="ExternalOutput").ap()

    eps_holder = {}

    def emit_squash(pool, s_ap, v_ap, npart):
        """v = s * msq/(1+msq)/(sqrt(msq+1e-8)+1e-8); per-partition over o.

        s_ap, v_ap: [npart, O]. All stats are [npart, 1]."""
        sq = pool.tile([P, O], f32, tag="sq_sq")
        nc.vector.tensor_mul(sq[:npart], s_ap, s_ap)
        msq = pool.tile([P, 1], f32, tag="sq_msq")
        nc.vector.reduce_sum(msq[:npart], sq[:npart], axis=AX.X)
        mag = pool.tile([P, 1], f32, tag="sq_mag")
        nc.scalar.activation(
            mag[:npart], msq[:npart], ACTF.Sqrt, bias=eps_holder["eps"][:npart]
        )
        magp = pool.tile([P, 1], f32, tag="sq_magp")
        nc.vector.tensor_scalar_add(magp[:npart], mag[:npart], 1e-8)
        den = pool.tile([P, 1], f32, tag="sq_den")
        # den = (msq + 1) * (mag + 1e-8)
        nc.vector.scalar_tensor_tensor(
            den[:npart], msq[:npart], 1.0, magp[:npart], ALU.add, ALU.mult
        )
        rec = pool.tile([P, 1], f32, tag="sq_rec")
        nc.vector.reciprocal_approx_fast(rec[:npart], den[:npart])
        fac = pool.tile([P, 1], f32, tag="sq_fac")
        nc.vector.tensor_mul(fac[:npart], msq[:npart], rec[:npart])
        nc.vector.tensor_scalar_mul(v_ap, s_ap, fac[:npart])

    with tile.TileContext(nc) as tc:
        import contextlib

        with contextlib.ExitStack() as ctx:
            singles = ctx.enter_context(tc.tile_pool(name="singles", bufs=1))
            stream = ctx.enter_context(tc.tile_pool(name="stream", bufs=2))
            upool = ctx.enter_context(tc.tile_pool(name="u", bufs=2))
            wpool = ctx.enter_context(tc.tile_pool(name="w", bufs=1))
            cbdp = ctx.enter_context(tc.tile_pool(name="cbdp", bufs=3))
            small = ctx.enter_context(tc.tile_pool(name="small", bufs=1))
            ping = ctx.enter_context(tc.tile_pool(name="ping", bufs=2))
            psum = ctx.enter_context(tc.tile_pool(name="psum", bufs=2, space="PSUM"))
            pvbcp = ctx.enter_context(tc.tile_pool(name="pvbcp", bufs=2, space="PSUM"))
            pupsum = ctx.enter_context(tc.tile_pool(name="pupsum", bufs=2, space="PSUM"))

            # ---- constants into SBUF
            # wp/xk split into 6 chunk-tiles so PE work starts after the first
            # chunk lands instead of after the whole 5.9MB load
            WCH = RB // 6  # 12
            wp_tiles = []
            xk_tiles = []
            for i in range(6):
                t = singles.tile([P, WCH, CO], mdt, tag=f"wp{i}")
                nc.sync.dma_start(
                    t, wp_d[i * WCH : (i + 1) * WCH].rearrange("a p f -> p a f")
                )
                wp_tiles.append(t)
                t2 = singles.tile([P, WCH, BL], mdt, tag=f"xk{i}")
                nc.sync.dma_start(
                    t2, xk_d[i * WCH : (i + 1) * WCH].rearrange("a p b -> p a b")
                )
                xk_tiles.append(t2)

            def wp_t(rb):
                return wp_tiles[rb // WCH][:, rb % WCH]
            e2_sb = singles.tile([P, NBT, P], mdt)
            nc.sync.dma_start(e2_sb, e2_d)
            e2c_sb = singles.tile([P, C, P], mdt)
            nc.sync.dma_start(e2c_sb, e2c_d)

            eps_sb = singles.tile([P, 1], f32)
            nc.vector.memset(eps_sb, 1e-8)
            eps_holder["eps"] = eps_sb

            dmask_sb = singles.tile([P, C, BT], f32)
            nc.sync.dma_start(dmask_sb, dmask_d)
            extmask_sb = singles.tile([C * BT, C, O], f32)
            nc.sync.dma_start(extmask_sb, extmask_d)
            v_full = singles.tile([P, CO], mdt)
            nc.vector.memset(v_full, 0.0)

            # ---- s0: uniform routing, dense K=9216 matmul
            ps0_full = psum.tile([C * BT, CO], f32, tag="ps")
            ps0 = ps0_full[:BL]
            for kc in range(RB):
                mm(
                    ps0,
                    xk_tiles[kc // WCH][:, kc % WCH],
                    wp_t(kc),
                    start=(kc == 0),
                    stop=(kc == RB - 1),
                )
            s0_sb = small.tile([BL, C, O], f32, tag="s0")
            nc.scalar.mul(s0_sb, ps0.rearrange("b (c o) -> b c o", o=O), 0.1)
            # squash s0 -> v_full[0:32] ([b, (c,o)] layout); per-(b,c) stats via
            # broadcast over o.
            sq0 = small.tile([BL, C, O], f32, tag="sq0")
            nc.vector.tensor_mul(sq0, s0_sb, s0_sb)
            msq0 = small.tile([BL, C], f32, tag="msq0")
            nc.vector.reduce_sum(msq0, sq0, axis=AX.X)
            mag0 = small.tile([BL, C], f32, tag="mag0")
            nc.scalar.activation(mag0, msq0, ACTF.Sqrt, bias=eps_sb[:BL])
            magp0 = small.tile([BL, C], f32, tag="magp0")
            nc.vector.tensor_scalar_add(magp0, mag0, 1e-8)
            den0 = small.tile([BL, C], f32, tag="den0")
            nc.vector.scalar_tensor_tensor(den0, msq0, 1.0, magp0, ALU.add, ALU.mult)
            rec0 = small.tile([BL, C], f32, tag="rec0")
            nc.vector.reciprocal_approx_fast(rec0, den0)
            fac0 = small.tile([BL, C], f32, tag="fac0")
            nc.vector.tensor_mul(fac0, msq0, rec0)
            nc.vector.tensor_tensor(
                v_full[:BL].rearrange("b (c o) -> b c o", o=O),
                s0_sb,
                fac0[:, :, None].to_broadcast((BL, C, O)),
                ALU.mult,
            )

            # ---- per-btile routing, chunk-pipelined over r-blocks.
            # Creation of btile bt+1 is emitted between it1 and it2 of btile
            # bt so its PE matmuls fill the DVE-heavy iteration phases.
            NCH = 4
            CHS = RB // NCH  # 18 r-blocks per chunk

            def emit_creation(bt):
                # block-diag matmuls; 3 r-blocks per 512-f32 PSUM bank (each
                # matmul inside one bank), grouped ACT copy per bank
                u_q = [
                    upool.tile([P, RB // 4, CO], mdt, tag=f"u{q}", name=f"u{q}_{bt}")
                    for q in range(4)
                ]

                for g6 in range(RB // 6):
                    xbd_sb = stream.tile([P, 6, P], mdt, tag="xbd")
                    nc.sync.dma_start(
                        xbd_sb,
                        xbd_d[bt, g6 * 6 : (g6 + 1) * 6].rearrange("g p f -> p g f"),
                    )
                    pu = pupsum.tile([P, 2, 512], f32, tag="pu")
                    for k in range(2):
                        rb0 = g6 * 6 + k * 3
                        for j in range(3):
                            mm(
                                pu[:, k, j * CO : (j + 1) * CO],
                                xbd_sb[:, k * 3 + j],
                                wp_t(rb0 + j),
                                start=True,
                                stop=True,
                            )
                        nc.scalar.copy(
                            u_q[rb0 // (RB // 4)][
                                :, rb0 % (RB // 4) : rb0 % (RB // 4) + 3
                            ],
                            pu[:, k, : 3 * CO],
                        )

                return u_q

            def emit_iter(bt, u_q, b_sb, v80, it):
                def u_t(rb):
                    return u_q[rb // (RB // 4)][:, rb % (RB // 4)]

                # vbc[(b,rs), (c,o)] = v[b,c,o] via tiny PE matmuls
                pvbc = pvbcp.tile([P, CO], f32, tag="pvbc")
                if it == 1:
                    mm(pvbc, e2_sb[:, bt], v_full, start=True, stop=True)
                else:
                    for c in range(C):
                        mm(
                            pvbc[:, c * O : (c + 1) * O],
                            e2c_sb[:, c],
                            v80,
                            start=True,
                            stop=True,
                        )
                if MM_DTYPE == "bf16":
                    vbcs = ping.tile([P, CO], mdt, tag="vbcs")
                    nc.scalar.copy(vbcs, pvbc)
                    vbc_src = vbcs
                else:
                    vbc_src = pvbc
                ps = psum.tile([C * BT, CO], f32, tag="ps")
                gdst = b_sb if it == 1 else ping.tile([P, RB, C], f32, tag="g2")
                for ch in range(NCH):
                    rbs = slice(ch * CHS, (ch + 1) * CHS)
                    # g = sum_o u * vbc for this chunk
                    w_sb = wpool.tile([P, CHS, CO], mdt, tag="w")
                    nc.vector.tensor_tensor(
                        w_sb,
                        u_q[ch],
                        vbc_src[:, None, :].to_broadcast((P, CHS, CO)),
                        ALU.mult,
                    )
                    nc.vector.reduce_sum(
                        gdst[:, rbs],
                        w_sb.rearrange("p a (c o) -> p a c o", o=O),
                        axis=AX.X,
                    )
                    if it == 2:
                        nc.vector.tensor_add(
                            b_sb[:, rbs], b_sb[:, rbs], gdst[:, rbs]
                        )
                    # softmax over c (no max subtraction; |b_ij| is small)
                    e_sb = ping.tile([P, CHS, C], f32, tag="esm")
                    nc.scalar.activation(e_sb, b_sb[:, rbs], ACTF.Exp)
                    ssum = ping.tile([P, CHS], f32, tag="ssum")
                    nc.vector.reduce_sum(ssum, e_sb, axis=AX.X)
                    rcp = ping.tile([P, CHS], f32, tag="rcp")
                    nc.vector.reciprocal_approx_fast(rcp, ssum)
                    c_sb = ping.tile([P, CHS, C], f32, tag="cij")
                    nc.gpsimd.tensor_tensor(
                        c_sb,
                        e_sb,
                        rcp[:, :, None].to_broadcast((P, CHS, C)),
                        ALU.mult,
                    )
                    # cbd[(b,rs), rb, c', b'] = c_ij * (b'==b) on GPSIMD
                    cbd_sb = cbdp.tile([P, CHS, C, BT], mdt, tag="cbd")
                    nc.gpsimd.tensor_tensor(
                        cbd_sb,
                        c_sb[:, :, :, None].to_broadcast((P, CHS, C, BT)),
                        dmask_sb[:, None, :, :].to_broadcast((P, CHS, C, BT)),
                        ALU.mult,
                    )
                    # s += sum_{r in chunk} c_ij * u  (PSUM accumulation)
                    cbd_flat = cbd_sb.rearrange("p a c b -> p a (c b)")
                    for j in range(CHS):
                        rb = ch * CHS + j
                        mm(
                            ps,
                            cbd_flat[:, j],
                            u_t(rb),
                            start=(rb == 0),
                            stop=(rb == RB - 1),
                        )
                # diagonal c'=c -> s80[(c,b), o]: mask-mult + reduce over c
                tdiag = small.tile([C * BT, C, O], f32, tag="tdiag")
                nc.vector.tensor_tensor(
                    tdiag,
                    ps.rearrange("p (c o) -> p c o", o=O),
                    extmask_sb,
                    ALU.mult,
                )
                s80 = small.tile([C * BT, O], f32, tag="s80")
                nc.vector.reduce_sum(
                    s80, tdiag.rearrange("p c o -> p o c"), axis=AX.X
                )
                v80n = ping.tile([P, O], mdt if it == 1 else f32, tag="v80")
                if it == 1:
                    nc.vector.memset(v80n, 0.0)
                emit_squash(small, s80[:], v80n[: C * BT], C * BT)
                if it == 2:
                    nc.sync.dma_start(vout_d[bt], v80n[: C * BT])
                return v80n

            u_next = emit_creation(0)
            for bt in range(NBT):
                u_cur = u_next
                b_sb = ping.tile([P, RB, C], f32, tag="bij")
                v80 = emit_iter(bt, u_cur, b_sb, None, 1)
                emit_iter(bt, u_cur, b_sb, v80, 2)
                if bt + 1 < NBT:
                    u_next = emit_creation(bt + 1)

    nc.compile()
    _MODULE_CACHE[key] = nc
    return nc


# ----------------------------------------------------------------------------
# public entry point
# ----------------------------------------------------------------------------
def kernel(x, W):
    x = np.asarray(x, dtype=np.float32)
    W = np.asarray(W, dtype=np.float32)

    wp, e2, e2c, dmask, extmask = _pack_shared(W)
    in_maps = []
    for core in range(NCORES):
        xbd, xk = _pack_core(x[core * BL : (core + 1) * BL])
        in_maps.append(
            {
                "xbd": xbd,
                "xk": xk,
                "wp": wp,
                "e2": e2,
                "e2c": e2c,
                "dmask": dmask,
                "extmask": extmask,
            }
        )

    nc = _build_module()

    from concourse.bass_utils import run_bass_kernel_spmd

    res = run_bass_kernel_spmd(nc, in_maps, core_ids=list(range(NCORES)))

    out = np.empty((B, C, O), np.float32)
    for core in range(NCORES):
        v80 = res.results[core]["vout"]  # [NBT, C*BT, O]
        vc = v80.reshape(NBT, C, BT, O).transpose(0, 2, 1, 3).reshape(BL, C, O)
        out[core * BL : (core + 1) * BL] = vc
    return out


if __name__ == "__main__":
    rng = np.random.default_rng(0)
    x = rng.standard_normal((B, R, I), dtype=np.float32)
    W = rng.standard_normal((R, C, O, I), dtype=np.float32)
    v = kernel(x, W)
    print("kernel output", v.shape, v.dtype, float(np.abs(v).max()))
